# revision 1
# baseline (speedup 1.0000x reference)
"""Distributed Trainium2 kernel for the AEN (attentive episodic network) problem.

Reference computation (shapes):
    support_vs = support @ Wv.T + bv                    [8192, 512]
    q_proto    = queries @ Wv.T + bv                    [8192, 512]
    support_ks = LN(support @ Wk.T + bk)                [8192, 512]
    queries_qs = LN(queries @ Wq.T + bq)                [8192, 512]
    scores     = queries_qs @ support_ks.T / sqrt(512)  [8192, 8192]
    affinity   = softmax(scores, axis=1)
    class_proto= affinity @ support_vs                  [8192, 512]
    returns (q_proto, class_proto)

Sharding: 2x4 grid.  Queries split in halves (4096/core) x support split in
quarters (2048/core).  Core c=(i,j) computes partial-softmax numerator
P_ij = exp(Qh_i K_j^T) V_j  [4096,512] and denominator d_ij = row-sums of
exp; the host combines  class[half i] = sum_j P_ij / sum_j d_ij + bv.
This removes the 8x-replicated support projection of the pure
query-sharded layout (the matmul cost model charges out_free_size x
0.42ns per accumulation step, so projections dominate): model PE busy
drops ~509us -> ~361us/core.

Cost-model facts this design is built around (instruction_cost_v2.rs):
  - matmul cost = out_free_size * PE_CYCLE(0.417ns) * cyc/row, independent
    of K: accumulation steps are the currency; [128,1]-out matmuls (row
    sums via ones-rhs) are free.
  - PE pstate ramps to 2.4GHz only after 3us of continuous busy: keep PE
    fed (EXEC queue depth 32).
  - DVE ~1.04ns/row + access; ACT ~0.83ns/row; both far under PE here.
  - DMA cannot touch PSUM; every PSUM tile occupies a full 2KB bank
    (8 banks): layout = 6 shared [128,512]f32 (proj/scores/AV rotate)
    + 1 transpose bank + 1 sums bank.
On-HW quirks respected (measured by a previous session on this fleet):
  ScalarE ops reading large f32 are pathologically slow -> exp() reads an
  fp16 staging copy of the raw scores; all other ACT inputs are 16-bit
  except the [128,1] LN sqrt (measured fine).

All matmuls bf16 with f32 PSUM accumulation.  Host adds bv (it commutes
with the affinity average since rows of affinity sum to 1), so V-path
bias matmuls are dropped.  gamma/beta are applied on-device only when
not (1, 0) — a build variant handles the general case.
"""

import os

import ml_dtypes
import numpy as np

D = 1024  # model dim
O = 512  # out dim
NCORES = 8
NQSH = 2  # query-half split
NSSH = 4  # support-quarter split
NQH = 8192 // NQSH  # 4096 queries per core
NSQ = 8192 // NSSH  # 2048 support rows per core
NQT = NQH // 128  # 32 query token tiles per core
NST = NSQ // 128  # 16 support token tiles per core
QB = 512  # query block (PSUM-bank sized attention unit)
NQB = NQH // QB  # 8 query blocks per core
NPQ = 8192 // NCORES // 128  # 8 q_proto tiles per core (distinct slice)
NDT = D // 128  # 8 contraction tiles
NOT = O // 128  # 4 outdim tiles
SCALE = 1.0 / float(np.sqrt(np.float32(O)))
LN_EPS = 1e-5
BF16 = ml_dtypes.bfloat16

_CACHE: dict = {}

LAST_RESULTS = None

# production graph variant (ablate flags baked into kernel())
PROD_ABLATE: tuple = ()


def _build_graph(reps=1, gb=False, ablate=()):
    """gb=True applies gamma/beta generally; False assumes (1, 0)."""
    ablate = set(ablate)
    import concourse.bass as bass  # noqa: F401
    import concourse.tile as tile
    from concourse import bacc, mybir
    from concourse.masks import make_identity

    f32 = mybir.dt.float32
    f16 = mybir.dt.float16
    bf16 = mybir.dt.bfloat16
    Alu = mybir.AluOpType
    Act = mybir.ActivationFunctionType

    nc = bacc.Bacc(
        "TRN2", target_bir_lowering=False, debug=False, num_devices=NCORES
    )

    sTp = nc.dram_tensor("sTp", [NST, 128, D], bf16, kind="ExternalInput").ap()
    qTp = nc.dram_tensor("qTp", [NQT, 128, D], bf16, kind="ExternalInput").ap()
    qPp = nc.dram_tensor("qPp", [NPQ, 128, D], bf16, kind="ExternalInput").ap()
    w = nc.dram_tensor("w", [D, 3 * O], bf16, kind="ExternalInput").ap()
    # bias rows [1, 1024] = [bq | bk], applied via K=1 matmul (bv -> host)
    brow = nc.dram_tensor("brow", [1, 2 * O], bf16, kind="ExternalInput").ap()
    if gb:
        g_p = nc.dram_tensor("g_p", [O, 1], f32, kind="ExternalInput").ap()
        be_p = nc.dram_tensor("be_p", [O, 1], f32, kind="ExternalInput").ap()
    out_part = nc.dram_tensor("out_part", [NQH, O], f32, kind="ExternalOutput").ap()
    if "rowsum" in ablate:
        out_sums = nc.dram_tensor("out_sums", [NQB, QB], f32, kind="ExternalOutput").ap()
    else:
        out_sums = nc.dram_tensor("out_sums", [128, 16 * NQB], f32, kind="ExternalOutput").ap()
    out_q = nc.dram_tensor("out_q", [NPQ * 128, O], f32, kind="ExternalOutput").ap()

    from contextlib import ExitStack

    with tile.TileContext(nc) as tc:
        with ExitStack() as ctx:
            ent = ctx.enter_context
            consts = ent(tc.tile_pool(name="consts", bufs=1))
            wp = ent(tc.tile_pool(name="wp", bufs=NDT))
            sp = ent(tc.tile_pool(name="sp", bufs=5))
            stp = ent(tc.tile_pool(name="stp", bufs=10))
            yp = ent(tc.tile_pool(name="yp", bufs=3))
            ktp = ent(tc.tile_pool(name="ktp", bufs=1))
            vvp = ent(tc.tile_pool(name="vvp", bufs=1))
            qqp = ent(tc.tile_pool(name="qqp", bufs=2))
            sfp = ent(tc.tile_pool(name="sfp", bufs=2))
            exl = ent(tc.tile_pool(name="exl", bufs=3))
            ocp = ent(tc.tile_pool(name="ocp", bufs=3))
            psB = ent(tc.tile_pool(name="psB", bufs=6, space="PSUM"))
            psT = ent(tc.tile_pool(name="psT", bufs=1, space="PSUM"))
            psS = ent(tc.tile_pool(name="psS", bufs=1, space="PSUM"))

            ident = consts.tile([128, 128], bf16, name="ident")
            make_identity(nc, ident)
            ones = consts.tile([128, 1], bf16, name="ones")
            nc.vector.memset(ones, 1.0)
            ones_row = consts.tile([1, 128], bf16, name="ones_row")
            nc.vector.memset(ones_row, 1.0)
            eps_t = consts.tile([128, 1], f32, name="eps_t")
            nc.vector.memset(eps_t, LN_EPS)
            brow_sb = consts.tile([1, 2 * O], bf16, name="brow_sb")
            nc.sync.dma_start(out=brow_sb, in_=brow)
            gam = []
            bet = []
            if gb:
                for j in range(NOT):
                    g_t = consts.tile([128, 1], f32, name=f"g{j}")
                    nc.sync.dma_start(out=g_t, in_=g_p[j * 128 : (j + 1) * 128, :])
                    gam.append(g_t)
                    b_t = consts.tile([128, 1], f32, name=f"b{j}")
                    nc.sync.dma_start(out=b_t, in_=be_p[j * 128 : (j + 1) * 128, :])
                    bet.append(b_t)

            wt = []
            for k in range(NDT):
                wtk = wp.tile([128, 3 * O], bf16, name=f"wt{k}", tag="wt")
                nc.sync.dma_start(out=wtk, in_=w[k * 128 : (k + 1) * 128, :])
                wt.append(wtk)

            def proj(ps, xt, off, bias):
                # one projection: 8 accumulating matmuls + optional K=1 bias
                for k in range(NDT):
                    nc.tensor.matmul(
                        ps, xt[:, k * 128 : (k + 1) * 128],
                        wt[k][:, off : off + O],
                        start=(k == 0), stop=(k == NDT - 1 and not bias),
                    )
                if bias:
                    nc.tensor.matmul(
                        ps, ones_row, brow_sb[:, off : off + O],
                        start=False, stop=True,
                    )

            def ln_norm(ps):
                # LN stats straight off PSUM; normalize to bf16
                y = yp.tile([128, O], bf16, name="y", tag="yp")
                if "ln" in ablate:
                    nc.vector.tensor_copy(y, ps)
                    return y
                stats = stp.tile([128, 6], f32, name="stats", tag="stp")
                nc.vector.bn_stats(stats, ps)
                mv = stp.tile([128, 2], f32, name="mv", tag="stp")
                nc.vector.bn_aggr(mv, stats)
                rstd = stp.tile([128, 1], f32, name="rstd", tag="stp")
                nc.scalar.activation(
                    rstd, mv[:, 1:2], Act.Sqrt, bias=eps_t, scale=1.0
                )
                nc.vector.reciprocal(rstd, rstd)
                nc.vector.tensor_scalar(
                    y, ps, mv[:, 0:1], rstd, Alu.subtract, Alu.mult
                )
                return y

            def pt_copy(src, dst, j):
                # post-transpose PSUM->SBUF copy, optionally applying
                # gamma/beta (per-partition scalars after the transpose)
                if gb:
                    if "actcopy" in ablate:
                        nc.scalar.activation(
                            dst, src, Act.Identity, bias=bet[j], scale=gam[j]
                        )
                    else:
                        nc.vector.tensor_scalar(
                            dst, src, gam[j], bet[j], Alu.mult, Alu.add
                        )
                else:
                    if "actcopy" in ablate:
                        nc.scalar.activation(dst, src, Act.Identity)
                    else:
                        nc.vector.tensor_copy(dst, src)

            def transpose_out(y, dst):
                # PE-transpose the 4 o-blocks into one PSUM bank, then copy
                # to SBUF (one wide op when gamma/beta don't apply)
                pt = psT.tile([128, O], bf16, name="pt", tag="psT")
                for j in range(NOT):
                    nc.tensor.transpose(
                        pt[:, j * 128 : (j + 1) * 128],
                        y[:, j * 128 : (j + 1) * 128],
                        ident,
                    )
                if gb:
                    for j in range(NOT):
                        pt_copy(
                            pt[:, j * 128 : (j + 1) * 128],
                            dst[:, j * 128 : (j + 1) * 128],
                            j,
                        )
                else:
                    pt_copy(pt, dst, 0)

            for _rep in range(reps):
                # K feature-major [o, s] (j-major within each 512-col block)
                # + V token-major; both SBUF-resident for the attention loop
                kT = ktp.tile([128, NST * O], bf16, name="kT", tag="ktp")
                vv = vvp.tile([128, NST * O], bf16, name="vv", tag="vvp")
                if "rowsum" in ablate:
                    sm_all = ocp.tile([1, NQB * QB], f32, name="sm_all", tag="smo")
                else:
                    sm_all = ocp.tile([128, 16 * NQB], f32, name="sm_all", tag="smo")

                # ---- support-quarter projections: K (LN'd, transposed) + V
                if "nosup" in ablate:
                    nc.gpsimd.memset(kT, 0.01)
                    nc.gpsimd.memset(vv, 0.01)
                for t in range(NST if "nosup" not in ablate else 0):
                    xt = sp.tile([128, D], bf16, name="xt", tag="sp")
                    nc.sync.dma_start(out=xt, in_=sTp[t])
                    ps_k = psB.tile([128, O], f32, name="ps_k", tag="psB")
                    ps_v = psB.tile([128, O], f32, name="ps_v", tag="psB")
                    proj(ps_k, xt, O, bias="bias" not in ablate)
                    proj(ps_v, xt, 2 * O, bias=False)
                    yk = ln_norm(ps_k)
                    transpose_out(yk, kT[:, t * O : (t + 1) * O])
                    nc.vector.tensor_copy(vv[:, t * O : (t + 1) * O], ps_v)

                # ---- all query projections up front: qq_all[j] holds the
                # LN'd, transposed query features for the whole half, so the
                # attention stream below never waits on a projection chain
                qq_all = [
                    qqp.tile([128, NQH], bf16, name=f"qqa{j}", tag=f"qq{j}")
                    for j in range(NOT)
                ]
                pend = []

                def emit_transp_q(m, yq):
                    pt = psT.tile([128, O], bf16, name="pt", tag="psT")
                    for j in range(NOT):
                        nc.tensor.transpose(
                            pt[:, j * 128 : (j + 1) * 128],
                            yq[:, j * 128 : (j + 1) * 128],
                            ident,
                        )
                    for j in range(NOT):
                        pt_copy(
                            pt[:, j * 128 : (j + 1) * 128],
                            qq_all[j][:, m * 128 : (m + 1) * 128],
                            j,
                        )

                for m in range(NQT if "noatt" not in ablate else 0):
                    xt = sp.tile([128, D], bf16, name="xt", tag="sp")
                    nc.sync.dma_start(out=xt, in_=qTp[m])
                    ps_q = psB.tile([128, O], f32, name="ps_q", tag="psB")
                    proj(ps_q, xt, 0, bias="bias" not in ablate)
                    yq = ln_norm(ps_q)
                    if pend:
                        emit_transp_q(*pend.pop())
                    pend.append((m, yq))
                if pend:
                    emit_transp_q(*pend.pop())

                # ---- partial attention per query block, lag-2 pipelined:
                # av_{t-2} is emitted after sc_t so PE never waits on the
                # stage+exp chain of the tile it just scored
                def emit_scores(qb):
                    sc = psB.tile([128, QB], f32, name="sc", tag="psB")
                    t = emit_scores.t
                    for j in range(NOT):
                        nc.tensor.matmul(
                            sc,
                            kT[:, t * O + j * 128 : t * O + (j + 1) * 128],
                            qq_all[j][:, qb * QB : (qb + 1) * QB],
                            start=(j == 0),
                            stop=(j == NOT - 1),
                        )
                    ex = exl.tile([128, QB], bf16, name="ex", tag="exl")
                    if "exp" in ablate:
                        nc.vector.tensor_copy(ex, sc)
                    elif "stage" in ablate:
                        nc.scalar.activation(ex, sc, Act.Exp, scale=SCALE)
                    else:
                        sch = sfp.tile([128, QB], f16, name="sch", tag="sfp")
                        if "gpstage" in ablate:
                            nc.gpsimd.tensor_copy(sch, sc)
                        else:
                            nc.vector.tensor_copy(sch, sc)
                        nc.scalar.activation(ex, sch, Act.Exp, scale=SCALE)
                    return ex

                for qb in range(NQB if "noatt" not in ablate else 0):
                    av = [
                        psB.tile([128, O], f32, name=f"av{qi}", tag="psB")
                        for qi in range(4)
                    ]
                    if "rowsum" in ablate:
                        sums = psS.tile([1, QB], f32, name="sums", tag="psS")
                    else:
                        sums = psS.tile([128, 16], f32, name="sums", tag="psS")

                    def emit_av(t, ex):
                        vsl = vv[:, t * O : (t + 1) * O]
                        if "rowsum" in ablate:
                            # one [1,512] matmul per t: sums over the 128
                            # support rows land per-query along the free dim
                            nc.tensor.matmul(
                                sums, ones[:, 0:1], ex,
                                start=(t == 0), stop=(t == NST - 1),
                            )
                        for qi in range(4):
                            exq = ex[:, qi * 128 : (qi + 1) * 128]
                            nc.tensor.matmul(
                                av[qi], exq, vsl,
                                start=(t == 0), stop=(t == NST - 1),
                            )
                            if "rowsum" in ablate:
                                continue
                            # start only on the first column: a matmul
                            # accumulation start zeroes the ENTIRE bank, so
                            # per-column starts would wipe sibling columns'
                            # t=0 contribution (the baseline kernel had this
                            # bug — its 1.4e-2 class error was mostly that)
                            nc.tensor.matmul(
                                sums[:, 4 * qi : 4 * qi + 1],
                                exq, ones,
                                start=(t == 0 and qi == 0),
                                stop=(t == NST - 1),
                            )

                    exs = []
                    for t in range(NST):
                        emit_scores.t = t
                        exs.append(emit_scores(qb))
                        if t >= 2:
                            emit_av(t - 2, exs[t - 2])
                    emit_av(NST - 2, exs[NST - 2])
                    emit_av(NST - 1, exs[NST - 1])
                    if "rowsum" in ablate:
                        nc.vector.tensor_copy(
                            sm_all[0:1, qb * QB : (qb + 1) * QB], sums
                        )
                    else:
                        nc.vector.tensor_copy(
                            sm_all[:, qb * 16 : (qb + 1) * 16], sums
                        )
                    for qi in range(4):
                        oc = ocp.tile([128, O], f32, name="oc", tag="ocp")
                        if "gpcopy" in ablate:
                            nc.gpsimd.tensor_copy(oc, av[qi])
                        else:
                            nc.vector.tensor_copy(oc, av[qi])
                        row = qb * QB + qi * 128
                        nc.sync.dma_start(
                            out=out_part[row : row + 128, :], in_=oc
                        )

                # ---- q_prototype for this core's distinct 1024-query slice
                for m in range(NPQ):
                    xt = sp.tile([128, D], bf16, name="xt", tag="sp")
                    nc.sync.dma_start(out=xt, in_=qPp[m])
                    ps_pv = psB.tile([128, O], f32, name="ps_pv", tag="psB")
                    proj(ps_pv, xt, 2 * O, bias=False)
                    oc = ocp.tile([128, O], f32, name="oc", tag="ocp")
                    nc.vector.tensor_copy(oc, ps_pv)
                    nc.sync.dma_start(
                        out=out_q[m * 128 : (m + 1) * 128, :], in_=oc
                    )

                # row-sum partials (decoded host-side)
                nc.sync.dma_start(out=out_sums, in_=sm_all)

    nc.compile()
    return nc


def _pack_fm(xT):
    # [D, N] feature-major -> [N/128, 128, D]: block (m, p, k*128+b) =
    # xT[k*128+p, m*128+b], so each SBUF load is one contiguous 2-D DMA and
    # xt[:, k*128:(k+1)*128] is the [d, tok] lhsT block for contraction tile k
    n = xT.shape[1]
    return np.ascontiguousarray(
        xT.reshape(NDT, 128, n // 128, 128).transpose(2, 1, 0, 3).reshape(n // 128, 128, D)
    )


def _prep_inputs(support_set, queries, Wq, bq, Wk, bk, Wv, bv, ln_gamma, ln_beta):
    gb = not (
        np.allclose(np.asarray(ln_gamma), 1.0)
        and np.allclose(np.asarray(ln_beta), 0.0)
    )
    sT = np.ascontiguousarray(np.asarray(support_set, np.float32).T).astype(BF16)
    qT = np.ascontiguousarray(np.asarray(queries, np.float32).T).astype(BF16)
    sTp = _pack_fm(sT)  # [64, 128, D]
    qTp = _pack_fm(qT)  # [64, 128, D]
    w_cat = np.ascontiguousarray(
        np.concatenate(
            [np.asarray(Wq).T, np.asarray(Wk).T, np.asarray(Wv).T], axis=1
        ).astype(np.float32)
    ).astype(BF16)
    brow = np.concatenate(
        [np.asarray(bq), np.asarray(bk)]
    ).astype(np.float32).reshape(1, 2 * O).astype(BF16)

    shared = {
        "w": w_cat,
        "brow": np.ascontiguousarray(brow),
    }
    if gb:
        shared["g_p"] = np.asarray(ln_gamma, np.float32).reshape(O, 1).copy()
        shared["be_p"] = np.asarray(ln_beta, np.float32).reshape(O, 1).copy()
    in_maps = []
    for c in range(NCORES):
        i, j = c // NSSH, c % NSSH
        m = dict(shared)
        m["sTp"] = np.ascontiguousarray(sTp[j * NST : (j + 1) * NST])
        m["qTp"] = np.ascontiguousarray(qTp[i * NQT : (i + 1) * NQT])
        m["qPp"] = np.ascontiguousarray(qTp[c * NPQ : (c + 1) * NPQ])
        in_maps.append(m)
    return in_maps, gb


def kernel(support_set, queries, Wq, bq, Wk, bk, Wv, bv, ln_gamma, ln_beta):
    global LAST_RESULTS
    from concourse.bass_utils import run_bass_kernel_spmd

    in_maps, gb = _prep_inputs(
        support_set, queries, Wq, bq, Wk, bk, Wv, bv, ln_gamma, ln_beta
    )
    key = ("nc", gb, PROD_ABLATE)
    if key not in _CACHE:
        _CACHE[key] = _build_graph(gb=gb, ablate=PROD_ABLATE)
    nc = _CACHE[key]
    _CACHE["in_maps"] = in_maps
    _CACHE["gb"] = gb
    res = run_bass_kernel_spmd(
        nc, in_maps, core_ids=list(range(NCORES)), trace=False
    )
    LAST_RESULTS = res

    bv32 = np.asarray(bv, np.float32)
    q_proto = np.concatenate(
        [np.asarray(res.results[c]["out_q"], np.float32) for c in range(NCORES)],
        axis=0,
    ) + bv32
    halves = []
    for i in range(NQSH):
        P = np.zeros((NQH, O), np.float64)
        S = 0.0
        for j in range(NSSH):
            r = res.results[i * NSSH + j]
            P += np.asarray(r["out_part"], np.float64)
            S = S + np.asarray(r["out_sums"], np.float64)
        if "rowsum" in PROD_ABLATE:
            d = S.reshape(NQH)
        else:
            d = S.reshape(128, NQB, 4, 4)[:, :, :, 0].transpose(1, 2, 0).reshape(NQH)
        halves.append(P / d[:, None] + bv32)
    c_proto = np.concatenate(halves, axis=0)
    return (
        np.asarray(q_proto, np.float32),
        np.asarray(c_proto, np.float32),
    )


def _bench_callable(nc):
    """Single-bind jitted callable over 8 cores with device-resident inputs."""
    import jax
    from jax.experimental.shard_map import shard_map
    from jax.sharding import Mesh, NamedSharding, PartitionSpec

    from concourse import bass2jax, mybir

    in_maps = _CACHE["in_maps"]

    partition_name = (
        nc.partition_id_tensor.name if nc.partition_id_tensor else None
    )
    in_names: list[str] = []
    out_names: list[str] = []
    out_avals = []
    zero_outs = []
    for alloc in nc.m.functions[0].allocations:
        if not isinstance(alloc, mybir.MemoryLocationSet):
            continue
        name = alloc.memorylocations[0].name
        if alloc.kind == "ExternalInput":
            if name != partition_name:
                in_names.append(name)
        elif alloc.kind == "ExternalOutput":
            shape = tuple(alloc.tensor_shape)
            dtype = mybir.dt.np(alloc.dtype)
            out_names.append(name)
            out_avals.append(jax.core.ShapedArray(shape, dtype))
            zero_outs.append(np.zeros(shape, dtype))
    n_params = len(in_names)
    in_names_full = list(in_names) + out_names
    if partition_name is not None:
        in_names_full.append(partition_name)

    def _body(*args):
        operands = list(args)
        if partition_name is not None:
            operands.append(bass2jax.partition_id_tensor())
        outs = bass2jax._bass_exec_p.bind(
            *operands,
            out_avals=tuple(out_avals),
            in_names=tuple(in_names_full),
            out_names=tuple(out_names),
            lowering_input_output_aliases=(),
            sim_require_finite=True,
            sim_require_nnan=True,
            nc=nc,
        )
        return tuple(outs)

    devices = jax.devices()[:NCORES]
    mesh = Mesh(np.asarray(devices), ("core",))
    n_outs = len(out_avals)
    in_specs = (PartitionSpec("core"),) * (n_params + n_outs)
    out_specs = (PartitionSpec("core"),) * n_outs
    sharded = jax.jit(
        shard_map(
            _body, mesh=mesh, in_specs=in_specs, out_specs=out_specs,
            check_rep=False,
        )
    )
    per_core = [
        [np.asarray(in_maps[c][name]) for name in in_names] for c in range(NCORES)
    ]
    concat_in = [
        np.concatenate([per_core[c][i] for c in range(NCORES)], axis=0)
        for i in range(n_params)
    ]
    concat_zeros = [
        np.zeros((NCORES * z.shape[0], *z.shape[1:]), z.dtype) for z in zero_outs
    ]
    sh = NamedSharding(mesh, PartitionSpec("core"))
    dev_in = [jax.device_put(a, sh) for a in concat_in]
    dev_zeros = [jax.device_put(a, sh) for a in concat_zeros]
    jax.block_until_ready(dev_in)
    jax.block_until_ready(dev_zeros)

    def run():
        out = sharded(*dev_in, *dev_zeros)
        jax.block_until_ready(out)
        return out

    return run


def benchmark(n_reps=5, timing_reps=8, **graph_kw):
    """Estimate per-execution device time (ns) by unrolling the kernel body
    n_reps times inside one NEFF and differencing against the 1-rep NEFF."""
    import time

    assert "in_maps" in _CACHE, "call kernel() first"
    graph_kw.setdefault("gb", _CACHE.get("gb", False))
    key1 = ("bnc", 1, tuple(sorted(graph_kw.items())))
    keyN = ("bnc", n_reps, tuple(sorted(graph_kw.items())))
    if key1 not in _CACHE:
        _CACHE[key1] = _build_graph(reps=1, **graph_kw)
    if keyN not in _CACHE:
        _CACHE[keyN] = _build_graph(reps=n_reps, **graph_kw)
    run1 = _bench_callable(_CACHE[key1])
    runN = _bench_callable(_CACHE[keyN])
    run1()
    runN()  # warm compiles

    def once(fn):
        t0 = time.perf_counter()
        fn()
        return time.perf_counter() - t0

    # axon RTT noise (tens of ms, in waves) dwarfs the device time, and it
    # is strictly additive — so take the MINIMUM wall over interleaved
    # trials for each NEFF independently: each min converges to its quiet
    # floor, and the floor difference is the device-side marginal.
    t1s, tNs = [], []
    for _ in range(max(timing_reps, 12)):
        t1s.append(once(run1))
        tNs.append(once(runN))
    t1, tN = float(np.min(t1s)), float(np.min(tNs))
    per_exec_s = (tN - t1) / (n_reps - 1)
    if per_exec_s <= 0:
        # pathological noise epoch: fall back to an async-pipelined chain
        # of the n_reps NEFF (device executions queue back-to-back, one
        # blocking sync at the end, so device time dominates the slope)
        import jax

        cells = {
            v: c.cell_contents
            for v, c in zip(runN.__code__.co_freevars, runN.__closure__)
        }
        sharded = cells["sharded"]
        dev_in, dev_zeros = cells["dev_in"], cells["dev_zeros"]

        def chain(k):
            t0 = time.perf_counter()
            outs = None
            for _ in range(k):
                outs = sharded(*dev_in, *dev_zeros)
            jax.block_until_ready(outs)
            return time.perf_counter() - t0

        chain(2)
        c1 = [chain(1) for _ in range(4)]
        cK = [chain(9) for _ in range(4)]
        per_exec_s = (min(cK) - min(c1)) / 8 / n_reps
    return per_exec_s * 1e9, t1, tN



# revision 3
# speedup vs baseline: 14.2839x; 14.2839x over previous
"""Distributed Trainium2 kernel for the AEN (attentive episodic network) problem.

Reference computation (shapes):
    support_vs = support @ Wv.T + bv                    [8192, 512]
    q_proto    = queries @ Wv.T + bv                    [8192, 512]
    support_ks = LN(support @ Wk.T + bk)                [8192, 512]
    queries_qs = LN(queries @ Wq.T + bq)                [8192, 512]
    scores     = queries_qs @ support_ks.T / sqrt(512)  [8192, 8192]
    affinity   = softmax(scores, axis=1)
    class_proto= affinity @ support_vs                  [8192, 512]
    returns (q_proto, class_proto)

Sharding: 2x4 grid. Queries split in halves (4096/core) x support split in
quarters (2048/core). Core (i,j) computes partial-softmax numerator
P_ij = exp(Qh_i K_j^T) V_j [4096,512] and denominator d_ij = row-sums of
exp; the host combines class[half i] = sum_j P_ij / sum_j d_ij + bv.

Projection dedup via on-device collectives (measured ~50us over computing
them redundantly):
  - K for quarter j is needed by the 2 cores sharing j: each projects 8 of
    the 16 tiles and a pair-AllGather of the LN'd, PE-transposed K
    (1MB bf16) shares them. The gather is kicked right after the 8 K tiles
    so it overlaps the V/Q/qP projections (~55us of PE work).
  - V stays fully local (16 tiles): a V-gather's latency costs more than
    the 13.6us of PE it would save.
  - Q for half i is needed by the 4 cores sharing i: each projects 8 of 32
    tiles, quad-AllGather (1MB).
  - Core mapping c = j*2+i makes the K-pairs adjacent cores; measured
    faster than the strided {j, j+4} pairing.

Attention per qb=512-query block over the quarter's 16 support tiles,
lag-2 pipelined so PE never waits on the exp of the tile it just scored.
exp() runs on ScalarE reading the scores PSUM f32 directly (a previous
session's f16-staging detour re-measured slower once the timing harness
was made drift-robust). Row sums via per-qi [128,1]-out matmuls whose
LDWEIGHTS hide under the AV streams (the [1,512] ones-stationary variant
re-measured neutral-to-worse).

All matmuls bf16 with f32 PSUM accumulation. Host adds bv (commutes with
the affinity average since affinity rows sum to 1). gamma/beta applied
on-device only when not (1, 0).

Cost-model/HW facts this design is built around:
  - matmul stream cost = out_free_size cycles at 2.4GHz (213ns per
    [128,512] accumulation step); LDWEIGHTS overlaps the previous stream.
  - PE clock ramps 1.2->2.4GHz after ~3.4us of sustained busy; keep fed.
  - ACT exp [128,512]: ~540-650ns; DVE PSUM-read copies ~650ns.
  - Pair AllGather 1MB ~10-26us, quad 1MB ~10-35us (topology-dependent),
    via HBM bounce buffers on the gpsimd queue; overlapped with PE work.
"""

import os

import ml_dtypes
import numpy as np

D = 1024  # model dim
O = 512  # out dim
NCORES = 8
NQSH = 2  # query-half split
NSSH = 4  # support-quarter split
NQH = 8192 // NQSH  # 4096 queries per core's half
NSQ = 8192 // NSSH  # 2048 support rows per core's quarter
NQT = NQH // 128  # 32 query tiles per half
NST = NSQ // 128  # 16 support tiles per quarter
NLT = NST // 2  # 8 locally-projected K tiles per core
NLQ = NQT // 4  # 8 locally-projected Q tiles per core
QB = 512  # query block (PSUM-bank sized attention unit)
NQB = NQH // QB  # 8 query blocks per core
NPQ = 8192 // NCORES // 128  # 8 q_proto tiles per core (distinct slice)
NDT = D // 128  # 8 contraction tiles
NOT = O // 128  # 4 outdim tiles
SCALE = 1.0 / float(np.sqrt(np.float32(O)))
LN_EPS = 1e-5
BF16 = ml_dtypes.bfloat16

_CACHE: dict = {}
LAST_RESULTS = None
PROD_ABLATE: tuple = ()

# c = j*2 + i: adjacent pair {2j, 2j+1} shares support quarter j;
# strided quad {i, i+2, i+4, i+6} shares query half i.
PAIRS = [[2 * j, 2 * j + 1] for j in range(4)]
QUADS = [[0, 2, 4, 6], [1, 3, 5, 7]]


def _core_of(i, j):
    return j * 2 + i


def _build_graph(reps=1, gb=False, ablate=()):
    """gb=True applies gamma/beta generally; False assumes (1, 0).

    ablate flags (for A/B benchmarking; production uses none):
      'nocc'     - no collectives, project everything redundantly
      'kvgather' - pair-gather V along with K instead of local V
      'nostage'  - stage scores through an f16 SBUF copy before exp
      'rowsum'   - [1,512] ones-stationary row-sum matmuls
    """
    ablate = set(ablate)
    import concourse.bass as bass  # noqa: F401
    import concourse.tile as tile
    from concourse import bacc, mybir
    from concourse.masks import make_identity

    f32 = mybir.dt.float32
    f16 = mybir.dt.float16
    bf16 = mybir.dt.bfloat16
    Alu = mybir.AluOpType
    Act = mybir.ActivationFunctionType

    nocc = "nocc" in ablate
    mode = "nocc" if nocc else ("kv" if "kvgather" in ablate else "kg")
    n_sup = NLT if mode == "kv" else NST
    n_q = NQT if nocc else NLQ

    nc = bacc.Bacc(
        "TRN2", target_bir_lowering=False, debug=False, num_devices=NCORES
    )

    sTp = nc.dram_tensor("sTp", [n_sup, 128, D], bf16, kind="ExternalInput").ap()
    if mode == "kg":
        sKp = nc.dram_tensor("sKp", [NLT, 128, D], bf16, kind="ExternalInput").ap()
    qTp = nc.dram_tensor("qTp", [n_q, 128, D], bf16, kind="ExternalInput").ap()
    qPp = nc.dram_tensor("qPp", [NPQ, 128, D], bf16, kind="ExternalInput").ap()
    w = nc.dram_tensor("w", [D, 3 * O], bf16, kind="ExternalInput").ap()
    # bias rows [1, 1024] = [bq | bk], applied via K=1 matmul (bv -> host)
    brow = nc.dram_tensor("brow", [1, 2 * O], bf16, kind="ExternalInput").ap()
    if gb:
        g_p = nc.dram_tensor("g_p", [O, 1], f32, kind="ExternalInput").ap()
        be_p = nc.dram_tensor("be_p", [O, 1], f32, kind="ExternalInput").ap()
    out_part = nc.dram_tensor("out_part", [NQH, O], f32, kind="ExternalOutput").ap()
    if "rowsum" in ablate:
        out_sums = nc.dram_tensor("out_sums", [NQB, QB], f32, kind="ExternalOutput").ap()
    else:
        out_sums = nc.dram_tensor(
            "out_sums", [128, 16 * NQB], f32, kind="ExternalOutput"
        ).ap()
    out_q = nc.dram_tensor("out_q", [NPQ * 128, O], f32, kind="ExternalOutput").ap()

    from contextlib import ExitStack

    with tile.TileContext(nc) as tc:
        with ExitStack() as ctx:
            ent = ctx.enter_context
            consts = ent(tc.tile_pool(name="consts", bufs=1))
            wp = ent(tc.tile_pool(name="wp", bufs=NDT))
            sp = ent(tc.tile_pool(name="sp", bufs=5))
            stp = ent(tc.tile_pool(name="stp", bufs=10))
            yp = ent(tc.tile_pool(name="yp", bufs=3))
            chp = ent(tc.tile_pool(name="chp", bufs=4))
            smp = ent(tc.tile_pool(name="smp", bufs=1))
            ktp = ent(tc.tile_pool(name="ktp", bufs=1))
            vvp = ent(tc.tile_pool(name="vvp", bufs=1))
            qqp = ent(tc.tile_pool(name="qqp", bufs=2))
            sfp = ent(tc.tile_pool(name="sfp", bufs=2))
            exl = ent(tc.tile_pool(name="exl", bufs=3))
            ocp = ent(tc.tile_pool(name="ocp", bufs=3))
            psB = ent(tc.tile_pool(name="psB", bufs=6, space="PSUM"))
            psT = ent(tc.tile_pool(name="psT", bufs=1, space="PSUM"))
            psS = ent(tc.tile_pool(name="psS", bufs=1, space="PSUM"))
            if not nocc:
                dramp = ent(tc.tile_pool(name="dramp", bufs=1, space="DRAM"))

            ident = consts.tile([128, 128], bf16, name="ident")
            make_identity(nc, ident)
            ones = consts.tile([128, 1], bf16, name="ones")
            nc.vector.memset(ones, 1.0)
            ones_row = consts.tile([1, 128], bf16, name="ones_row")
            nc.vector.memset(ones_row, 1.0)
            eps_t = consts.tile([128, 1], f32, name="eps_t")
            nc.vector.memset(eps_t, LN_EPS)
            brow_sb = consts.tile([1, 2 * O], bf16, name="brow_sb")
            nc.sync.dma_start(out=brow_sb, in_=brow)
            gam = []
            bet = []
            if gb:
                for j in range(NOT):
                    g_t = consts.tile([128, 1], f32, name=f"g{j}")
                    nc.sync.dma_start(out=g_t, in_=g_p[j * 128 : (j + 1) * 128, :])
                    gam.append(g_t)
                    b_t = consts.tile([128, 1], f32, name=f"b{j}")
                    nc.sync.dma_start(out=b_t, in_=be_p[j * 128 : (j + 1) * 128, :])
                    bet.append(b_t)

            wt = []
            for k in range(NDT):
                wtk = wp.tile([128, 3 * O], bf16, name=f"wt{k}", tag="wt")
                nc.sync.dma_start(out=wtk, in_=w[k * 128 : (k + 1) * 128, :])
                wt.append(wtk)

            def proj(ps, xt, off, bias):
                # one projection: 8 accumulating matmuls + optional K=1 bias
                for k in range(NDT):
                    nc.tensor.matmul(
                        ps, xt[:, k * 128 : (k + 1) * 128],
                        wt[k][:, off : off + O],
                        start=(k == 0), stop=(k == NDT - 1 and not bias),
                    )
                if bias:
                    nc.tensor.matmul(
                        ps, ones_row, brow_sb[:, off : off + O],
                        start=False, stop=True,
                    )

            def ln_norm(ps):
                # LN stats straight off PSUM; normalize to bf16
                y = yp.tile([128, O], bf16, name="y", tag="yp")
                stats = stp.tile([128, 6], f32, name="stats", tag="stp")
                nc.vector.bn_stats(stats, ps)
                mv = stp.tile([128, 2], f32, name="mv", tag="stp")
                nc.vector.bn_aggr(mv, stats)
                rstd = stp.tile([128, 1], f32, name="rstd", tag="stp")
                nc.scalar.activation(
                    rstd, mv[:, 1:2], Act.Sqrt, bias=eps_t, scale=1.0
                )
                nc.vector.reciprocal(rstd, rstd)
                nc.vector.tensor_scalar(
                    y, ps, mv[:, 0:1], rstd, Alu.subtract, Alu.mult
                )
                return y

            def pt_copy(src, dst, j):
                # post-transpose PSUM->SBUF copy, optionally applying
                # gamma/beta (per-partition scalars after the transpose)
                if gb:
                    nc.vector.tensor_scalar(
                        dst, src, gam[j], bet[j], Alu.mult, Alu.add
                    )
                else:
                    nc.vector.tensor_copy(dst, src)

            def transpose_out(y, dst):
                # PE-transpose the 4 o-blocks into one PSUM bank, then copy
                # to SBUF (one wide op when gamma/beta don't apply)
                pt = psT.tile([128, O], bf16, name="pt", tag="psT")
                for j in range(NOT):
                    nc.tensor.transpose(
                        pt[:, j * 128 : (j + 1) * 128],
                        y[:, j * 128 : (j + 1) * 128],
                        ident,
                    )
                if gb:
                    for j in range(NOT):
                        pt_copy(
                            pt[:, j * 128 : (j + 1) * 128],
                            dst[:, j * 128 : (j + 1) * 128],
                            j,
                        )
                else:
                    pt_copy(pt, dst, 0)

            for _rep in range(reps):
                kT = ktp.tile([128, NST * O], bf16, name="kT", tag="ktp")
                vv = vvp.tile([128, NST * O], bf16, name="vv", tag="vvp")
                if "rowsum" in ablate:
                    sm_all = smp.tile([1, NQB * QB], f32, name="sm_all", tag="smo")
                else:
                    sm_all = smp.tile([128, 16 * NQB], f32, name="sm_all", tag="smo")

                if mode == "kv":
                    kvb = dramp.tile([256, NLT * O], bf16, name="kvb", tag="kvb")
                    kvg = dramp.tile([512, NLT * O], bf16, name="kvg", tag="kvg")
                elif mode == "kg":
                    kvb = dramp.tile([128, NLT * O], bf16, name="kvb", tag="kvb")
                    kvg = dramp.tile([256, NLT * O], bf16, name="kvg", tag="kvg")

                if mode == "kg":
                    # K projections for the local 8 tiles only; straight to
                    # the bounce so the pair-gather starts ASAP
                    for t in range(NLT):
                        xt = sp.tile([128, D], bf16, name="xt", tag="sp")
                        nc.sync.dma_start(out=xt, in_=sKp[t])
                        ps_k = psB.tile([128, O], f32, name="ps_k", tag="psB")
                        proj(ps_k, xt, O, bias=True)
                        yk = ln_norm(ps_k)
                        kch = chp.tile([128, O], bf16, name="kch", tag="chp")
                        transpose_out(yk, kch)
                        nc.sync.dma_start(
                            out=kvb[:, t * O : (t + 1) * O], in_=kch
                        )
                    nc.gpsimd.collective_compute(
                        "AllGather",
                        mybir.AluOpType.bypass,
                        replica_groups=PAIRS,
                        ins=[kvb.opt()],
                        outs=[kvg.opt()],
                    )
                    # V projections over the full quarter, SBUF-resident
                    for t in range(NST):
                        xt = sp.tile([128, D], bf16, name="xt", tag="sp")
                        nc.sync.dma_start(out=xt, in_=sTp[t])
                        ps_v = psB.tile([128, O], f32, name="ps_v", tag="psB")
                        proj(ps_v, xt, 2 * O, bias=False)
                        nc.vector.tensor_copy(vv[:, t * O : (t + 1) * O], ps_v)
                else:
                    for t in range(n_sup):
                        xt = sp.tile([128, D], bf16, name="xt", tag="sp")
                        nc.sync.dma_start(out=xt, in_=sTp[t])
                        ps_k = psB.tile([128, O], f32, name="ps_k", tag="psB")
                        ps_v = psB.tile([128, O], f32, name="ps_v", tag="psB")
                        proj(ps_k, xt, O, bias=True)
                        proj(ps_v, xt, 2 * O, bias=False)
                        yk = ln_norm(ps_k)
                        if nocc:
                            transpose_out(yk, kT[:, t * O : (t + 1) * O])
                            nc.vector.tensor_copy(
                                vv[:, t * O : (t + 1) * O], ps_v
                            )
                        else:
                            kch = chp.tile([128, O], bf16, name="kch", tag="chp")
                            transpose_out(yk, kch)
                            nc.sync.dma_start(
                                out=kvb[0:128, t * O : (t + 1) * O], in_=kch
                            )
                            vch = chp.tile([128, O], bf16, name="vch", tag="chp")
                            nc.vector.tensor_copy(vch, ps_v)
                            nc.sync.dma_start(
                                out=kvb[128:256, t * O : (t + 1) * O], in_=vch
                            )
                    if mode == "kv":
                        nc.gpsimd.collective_compute(
                            "AllGather",
                            mybir.AluOpType.bypass,
                            replica_groups=PAIRS,
                            ins=[kvb.opt()],
                            outs=[kvg.opt()],
                        )

                # ---- local query projections (LN'd, transposed)
                qq_all = [
                    qqp.tile([128, NQH], bf16, name=f"qqa{j}", tag=f"qq{j}")
                    for j in range(NOT)
                ]
                if not nocc:
                    qb_ = dramp.tile([512, NLQ * 128], bf16, name="qb_", tag="qb_")
                    qg = dramp.tile([2048, NLQ * 128], bf16, name="qg", tag="qg")
                pend = []

                def emit_transp_q(m, yq):
                    pt = psT.tile([128, O], bf16, name="pt", tag="psT")
                    for j in range(NOT):
                        nc.tensor.transpose(
                            pt[:, j * 128 : (j + 1) * 128],
                            yq[:, j * 128 : (j + 1) * 128],
                            ident,
                        )
                    if nocc:
                        for j in range(NOT):
                            pt_copy(
                                pt[:, j * 128 : (j + 1) * 128],
                                qq_all[j][:, m * 128 : (m + 1) * 128],
                                j,
                            )
                    else:
                        qch = chp.tile([128, O], bf16, name="qch", tag="chp")
                        if gb:
                            for j in range(NOT):
                                pt_copy(
                                    pt[:, j * 128 : (j + 1) * 128],
                                    qch[:, j * 128 : (j + 1) * 128],
                                    j,
                                )
                        else:
                            pt_copy(pt, qch, 0)
                        for j in range(NOT):
                            nc.sync.dma_start(
                                out=qb_[
                                    j * 128 : (j + 1) * 128,
                                    m * 128 : (m + 1) * 128,
                                ],
                                in_=qch[:, j * 128 : (j + 1) * 128],
                            )

                for m in range(n_q):
                    xt = sp.tile([128, D], bf16, name="xt", tag="sp")
                    nc.sync.dma_start(out=xt, in_=qTp[m])
                    ps_q = psB.tile([128, O], f32, name="ps_q", tag="psB")
                    proj(ps_q, xt, 0, bias=True)
                    yq = ln_norm(ps_q)
                    if pend:
                        emit_transp_q(*pend.pop())
                    pend.append((m, yq))
                if pend:
                    emit_transp_q(*pend.pop())

                if not nocc:
                    nc.gpsimd.collective_compute(
                        "AllGather",
                        mybir.AluOpType.bypass,
                        replica_groups=QUADS,
                        ins=[qb_.opt()],
                        outs=[qg.opt()],
                    )

                # ---- q_prototype for this core's distinct 1024-query slice
                # (fills PE while the gathers run)
                for m in range(NPQ):
                    xt = sp.tile([128, D], bf16, name="xt", tag="sp")
                    nc.sync.dma_start(out=xt, in_=qPp[m])
                    ps_pv = psB.tile([128, O], f32, name="ps_pv", tag="psB")
                    proj(ps_pv, xt, 2 * O, bias=False)
                    oc = ocp.tile([128, O], f32, name="oc", tag="ocp")
                    nc.vector.tensor_copy(oc, ps_pv)
                    nc.sync.dma_start(
                        out=out_q[m * 128 : (m + 1) * 128, :], in_=oc
                    )

                # ---- assemble gathered K/Q into SBUF
                if mode == "kg":
                    half = NLT * O
                    nc.sync.dma_start(out=kT[:, 0:half], in_=kvg[0:128, :])
                    nc.sync.dma_start(out=kT[:, half:], in_=kvg[128:256, :])
                elif mode == "kv":
                    half = NLT * O
                    nc.sync.dma_start(out=kT[:, 0:half], in_=kvg[0:128, :])
                    nc.sync.dma_start(out=kT[:, half:], in_=kvg[256:384, :])
                    nc.sync.dma_start(out=vv[:, 0:half], in_=kvg[128:256, :])
                    nc.sync.dma_start(out=vv[:, half:], in_=kvg[384:512, :])
                if not nocc:
                    qs = NLQ * 128
                    for j in range(NOT):
                        for r in range(4):
                            nc.sync.dma_start(
                                out=qq_all[j][:, r * qs : (r + 1) * qs],
                                in_=qg[r * 512 + j * 128 : r * 512 + (j + 1) * 128, :],
                            )

                # ---- partial attention per query block, lag-2 pipelined:
                # av_{t-2} is emitted after sc_t so PE never waits on the
                # exp of the tile it just scored
                def emit_scores(qb, t):
                    sc = psB.tile([128, QB], f32, name="sc", tag="psB")
                    for j in range(NOT):
                        nc.tensor.matmul(
                            sc,
                            kT[:, t * O + j * 128 : t * O + (j + 1) * 128],
                            qq_all[j][:, qb * QB : (qb + 1) * QB],
                            start=(j == 0),
                            stop=(j == NOT - 1),
                        )
                    ex = exl.tile([128, QB], bf16, name="ex", tag="exl")
                    if "nostage" in ablate:
                        sch = sfp.tile([128, QB], f16, name="sch", tag="sfp")
                        nc.vector.tensor_copy(sch, sc)
                        nc.scalar.activation(ex, sch, Act.Exp, scale=SCALE)
                    else:
                        nc.scalar.activation(ex, sc, Act.Exp, scale=SCALE)
                    return ex

                for qb in range(NQB):
                    av = [
                        psB.tile([128, O], f32, name=f"av{qi}", tag="psB")
                        for qi in range(4)
                    ]
                    if "rowsum" in ablate:
                        sums = psS.tile([1, QB], f32, name="sums", tag="psS")
                    else:
                        sums = psS.tile([128, 16], f32, name="sums", tag="psS")

                    def emit_av(t, ex):
                        vsl = vv[:, t * O : (t + 1) * O]
                        if "rowsum" in ablate:
                            nc.tensor.matmul(
                                sums, ones[:, 0:1], ex,
                                start=(t == 0), stop=(t == NST - 1),
                            )
                        for qi in range(4):
                            exq = ex[:, qi * 128 : (qi + 1) * 128]
                            nc.tensor.matmul(
                                av[qi], exq, vsl,
                                start=(t == 0), stop=(t == NST - 1),
                            )
                            if "rowsum" in ablate:
                                continue
                            # start only on the first column: a matmul
                            # accumulation start zeroes the ENTIRE bank, so
                            # per-column starts would wipe sibling columns'
                            # t=0 contribution
                            nc.tensor.matmul(
                                sums[:, 4 * qi : 4 * qi + 1],
                                exq, ones,
                                start=(t == 0 and qi == 0),
                                stop=(t == NST - 1),
                            )

                    exs = []
                    for t in range(NST):
                        exs.append(emit_scores(qb, t))
                        if t >= 2:
                            emit_av(t - 2, exs[t - 2])
                    emit_av(NST - 2, exs[NST - 2])
                    emit_av(NST - 1, exs[NST - 1])
                    if "rowsum" in ablate:
                        nc.vector.tensor_copy(
                            sm_all[0:1, qb * QB : (qb + 1) * QB], sums
                        )
                    else:
                        nc.vector.tensor_copy(
                            sm_all[:, qb * 16 : (qb + 1) * 16], sums
                        )
                    for qi in range(4):
                        oc = ocp.tile([128, O], f32, name="oc", tag="ocp")
                        nc.vector.tensor_copy(oc, av[qi])
                        row = qb * QB + qi * 128
                        nc.sync.dma_start(
                            out=out_part[row : row + 128, :], in_=oc
                        )

                # row-sum partials (decoded host-side)
                nc.sync.dma_start(out=out_sums, in_=sm_all)

    nc.compile()
    return nc


def _pack_fm(xT):
    # [D, N] feature-major -> [N/128, 128, D]: block (m, p, k*128+b) =
    # xT[k*128+p, m*128+b], so each SBUF load is one contiguous 2-D DMA and
    # xt[:, k*128:(k+1)*128] is the [d, tok] lhsT block for contraction tile k
    n = xT.shape[1]
    return np.ascontiguousarray(
        xT.reshape(NDT, 128, n // 128, 128).transpose(2, 1, 0, 3).reshape(n // 128, 128, D)
    )


def _prep_inputs(
    support_set, queries, Wq, bq, Wk, bk, Wv, bv, ln_gamma, ln_beta, mode="kg"
):
    gb = not (
        np.allclose(np.asarray(ln_gamma), 1.0)
        and np.allclose(np.asarray(ln_beta), 0.0)
    )
    sT = np.ascontiguousarray(np.asarray(support_set, np.float32).T).astype(BF16)
    qT = np.ascontiguousarray(np.asarray(queries, np.float32).T).astype(BF16)
    sTp = _pack_fm(sT)  # [64, 128, D]
    qTp = _pack_fm(qT)  # [64, 128, D]
    w_cat = np.ascontiguousarray(
        np.concatenate(
            [np.asarray(Wq).T, np.asarray(Wk).T, np.asarray(Wv).T], axis=1
        ).astype(np.float32)
    ).astype(BF16)
    brow = np.concatenate(
        [np.asarray(bq), np.asarray(bk)]
    ).astype(np.float32).reshape(1, 2 * O).astype(BF16)

    shared = {"w": w_cat, "brow": np.ascontiguousarray(brow)}
    if gb:
        shared["g_p"] = np.asarray(ln_gamma, np.float32).reshape(O, 1).copy()
        shared["be_p"] = np.asarray(ln_beta, np.float32).reshape(O, 1).copy()
    in_maps = [None] * NCORES
    for i in range(NQSH):
        for j in range(NSSH):
            c = _core_of(i, j)
            m = dict(shared)
            if mode == "nocc":
                m["sTp"] = np.ascontiguousarray(sTp[j * NST : (j + 1) * NST])
                m["qTp"] = np.ascontiguousarray(qTp[i * NQT : (i + 1) * NQT])
            else:
                base = j * NST + i * NLT
                if mode == "kg":
                    m["sTp"] = np.ascontiguousarray(sTp[j * NST : (j + 1) * NST])
                    m["sKp"] = np.ascontiguousarray(sTp[base : base + NLT])
                else:
                    m["sTp"] = np.ascontiguousarray(sTp[base : base + NLT])
                qbase = i * NQT + j * NLQ
                m["qTp"] = np.ascontiguousarray(qTp[qbase : qbase + NLQ])
            m["qPp"] = np.ascontiguousarray(qTp[c * NPQ : (c + 1) * NPQ])
            in_maps[c] = m
    return in_maps, gb


def _mode_of(ablate):
    return (
        "nocc" if "nocc" in ablate
        else ("kv" if "kvgather" in ablate else "kg")
    )


def kernel(support_set, queries, Wq, bq, Wk, bk, Wv, bv, ln_gamma, ln_beta):
    global LAST_RESULTS
    from concourse.bass_utils import run_bass_kernel_spmd

    in_maps, gb = _prep_inputs(
        support_set, queries, Wq, bq, Wk, bk, Wv, bv, ln_gamma, ln_beta,
        mode=_mode_of(PROD_ABLATE),
    )
    key = ("nc", gb, PROD_ABLATE)
    if key not in _CACHE:
        _CACHE[key] = _build_graph(gb=gb, ablate=PROD_ABLATE)
    nc = _CACHE[key]
    _CACHE["in_maps"] = in_maps
    _CACHE["gb"] = gb
    res = run_bass_kernel_spmd(
        nc, in_maps, core_ids=list(range(NCORES)), trace=False
    )
    LAST_RESULTS = res

    bv32 = np.asarray(bv, np.float32)
    q_proto = np.empty((8192, O), np.float32)
    for c in range(NCORES):
        q_proto[c * NPQ * 128 : (c + 1) * NPQ * 128] = np.asarray(
            res.results[c]["out_q"], np.float32
        )
    q_proto += bv32
    halves = []
    for i in range(NQSH):
        P = np.zeros((NQH, O), np.float64)
        S = 0.0
        for j in range(NSSH):
            r = res.results[_core_of(i, j)]
            P += np.asarray(r["out_part"], np.float64)
            S = S + np.asarray(r["out_sums"], np.float64)
        if "rowsum" in PROD_ABLATE:
            d = S.reshape(NQH)
        else:
            d = S.reshape(128, NQB, 4, 4)[:, :, :, 0].transpose(1, 2, 0).reshape(NQH)
        halves.append(P / d[:, None] + bv32)
    c_proto = np.concatenate(halves, axis=0)
    return (
        np.asarray(q_proto, np.float32),
        np.asarray(c_proto, np.float32),
    )


def _bench_callable(nc):
    """Single-bind jitted callable over 8 cores with device-resident inputs."""
    import jax
    from jax.experimental.shard_map import shard_map
    from jax.sharding import Mesh, NamedSharding, PartitionSpec

    from concourse import bass2jax, mybir

    in_maps = _CACHE["in_maps"]

    partition_name = (
        nc.partition_id_tensor.name if nc.partition_id_tensor else None
    )
    in_names: list[str] = []
    out_names: list[str] = []
    out_avals = []
    zero_outs = []
    for alloc in nc.m.functions[0].allocations:
        if not isinstance(alloc, mybir.MemoryLocationSet):
            continue
        name = alloc.memorylocations[0].name
        if alloc.kind == "ExternalInput":
            if name != partition_name:
                in_names.append(name)
        elif alloc.kind == "ExternalOutput":
            shape = tuple(alloc.tensor_shape)
            dtype = mybir.dt.np(alloc.dtype)
            out_names.append(name)
            out_avals.append(jax.core.ShapedArray(shape, dtype))
            zero_outs.append(np.zeros(shape, dtype))
    n_params = len(in_names)
    in_names_full = list(in_names) + out_names
    if partition_name is not None:
        in_names_full.append(partition_name)

    def _body(*args):
        operands = list(args)
        if partition_name is not None:
            operands.append(bass2jax.partition_id_tensor())
        outs = bass2jax._bass_exec_p.bind(
            *operands,
            out_avals=tuple(out_avals),
            in_names=tuple(in_names_full),
            out_names=tuple(out_names),
            lowering_input_output_aliases=(),
            sim_require_finite=True,
            sim_require_nnan=True,
            nc=nc,
        )
        return tuple(outs)

    devices = jax.devices()[:NCORES]
    mesh = Mesh(np.asarray(devices), ("core",))
    n_outs = len(out_avals)
    in_specs = (PartitionSpec("core"),) * (n_params + n_outs)
    out_specs = (PartitionSpec("core"),) * n_outs
    sharded = jax.jit(
        shard_map(
            _body, mesh=mesh, in_specs=in_specs, out_specs=out_specs,
            check_rep=False,
        )
    )
    per_core = [
        [np.asarray(in_maps[c][name]) for name in in_names] for c in range(NCORES)
    ]
    concat_in = [
        np.concatenate([per_core[c][i] for c in range(NCORES)], axis=0)
        for i in range(n_params)
    ]
    concat_zeros = [
        np.zeros((NCORES * z.shape[0], *z.shape[1:]), z.dtype) for z in zero_outs
    ]
    sh = NamedSharding(mesh, PartitionSpec("core"))
    dev_in = [jax.device_put(a, sh) for a in concat_in]
    dev_zeros = [jax.device_put(a, sh) for a in concat_zeros]
    jax.block_until_ready(dev_in)
    jax.block_until_ready(dev_zeros)

    def run():
        out = sharded(*dev_in, *dev_zeros)
        jax.block_until_ready(out)
        return out

    return run


def _chain_fn(nc):
    """Compile nc and return chain(k): k async executes + one blocking sync."""
    import time

    import jax

    run = _bench_callable(nc)
    cells = {
        v: c.cell_contents
        for v, c in zip(run.__code__.co_freevars, run.__closure__)
    }
    sharded = cells["sharded"]
    dev_in, dev_zeros = cells["dev_in"], cells["dev_zeros"]

    def chain(k):
        t0 = time.perf_counter()
        outs = None
        for _ in range(k):
            outs = sharded(*dev_in, *dev_zeros)
        jax.block_until_ready(outs)
        return time.perf_counter() - t0

    return chain


def benchmark(n_reps=17, timing_reps=16, **graph_kw):
    """Per-execution device time (ns) of the kernel body.

    Method: unroll the body R=n_reps times inside one NEFF; measure the
    wall of K chained executes of the R-rep NEFF and of the 1-rep NEFF
    back-to-back, compute the slope (tR - t1)/((R-1)*K) per round, and
    take the median over timing_reps interleaved rounds. The amplification
    makes the device-time difference (~(R-1)*K bodies) large against the
    tens-of-ms axon RTT noise, and the per-round differencing + median
    cancels the slow drift of that noise. (The previous min-of-walls
    differencing estimator had errors larger than the quantity itself —
    it reported anywhere from 0.1x to 25x the true body time across
    sessions.)
    """
    assert "in_maps" in _CACHE, "call kernel() first"
    graph_kw.setdefault("gb", _CACHE.get("gb", False))
    graph_kw.setdefault("ablate", PROD_ABLATE)
    R, K = n_reps, 4
    chains = {}
    for reps in (1, R):
        key = ("bnc", reps, tuple(sorted(graph_kw.items())))
        if key not in _CACHE:
            _CACHE[key] = _build_graph(reps=reps, **graph_kw)
        chains[reps] = _chain_fn(_CACHE[key])
    chains[1](2)
    chains[R](2)  # warm
    slopes = []
    t1s, tRs = [], []
    for _ in range(max(timing_reps, 8)):
        t1 = chains[1](K)
        tR = chains[R](K)
        t1s.append(t1)
        tRs.append(tR)
        slopes.append((tR - t1) / ((R - 1) * K))
    med = float(np.median(slopes))
    return med * 1e9, float(np.min(t1s)), float(np.min(tRs))
